# revision 1
# baseline (speedup 1.0000x reference)
"""CrossFrameAttention Trainium2 kernel.

Full (unsharded) inputs -> full output. Internally: data-parallel over the
fused frame*batch dim (F*B = 8 elements, one per NeuronCore), weights
replicated. Per core, a fused 1x1-conv QKV projection + softmax attention
written in Bass/Tile.

The wall-clock of a warm run is dominated by host-side costs (per-call jit
recompile + axon tunnel transfers), so the kernel is shaped around those:
  - the JAX persistent compilation cache is enabled so repeat runs skip the
    XLA+NEFF recompile entirely;
  - all device I/O is fp16 (x, weights in; attention output out), halving
    tunnel bytes;
  - the cheap epilogue (gamma * (attn + bv) + x residual) runs on the host
    in fp32, which also removes every PE transpose from the device kernel
    (output leaves the device in [N, C] layout).

Self-contained: hardcodes shapes from the problem spec.
"""

import numpy as np

import jax

# Warm runs create fresh jax.jit objects inside run_bass_kernel_spmd; the
# persistent cache turns their XLA+NEFF recompiles into disk hits.
jax.config.update("jax_compilation_cache_dir", "/tmp/jax_bass_cc")
jax.config.update("jax_persistent_cache_min_compile_time_secs", 0.0)
jax.config.update("jax_persistent_cache_min_entry_size_bytes", -1)

F, B, C, HH, WW = 4, 2, 256, 64, 64
N = HH * WW            # 4096 tokens per (frame,batch) element
FB = F * B             # 8 == n_cores
DQK = 32               # q/k channel dim (C/8)
NBLK = N // 512        # 8 query blocks of 512
NJ = N // 128          # 32 key chunks of 128

_CACHE = {}


def _build_nc():
    import concourse.mybir as mybir
    from concourse import bacc
    from concourse.tile import TileContext

    f32 = mybir.dt.float32
    f16 = mybir.dt.float16
    bf16 = mybir.dt.bfloat16
    AF = mybir.ActivationFunctionType
    ALU = mybir.AluOpType

    nc = bacc.Bacc(None, target_bir_lowering=False, debug=False)

    # single packed fp16 input: [x | WqT Wk T | WvT | bqk] along columns —
    # one host->device tensor per call instead of four (fewer per-array RPCs)
    PACK = N + 2 * DQK + C + 1
    x_d = nc.dram_tensor("pack", [C, PACK], f16, kind="ExternalInput")
    wqk_d = x_d[:, N:N + 2 * DQK]
    wv_d = x_d[:, N + 2 * DQK:N + 2 * DQK + C]
    bqk_d = x_d[:, PACK - 1:PACK]
    # attention output, [token, channel] layout (transposed on host), int8
    # with one fp32 scale per token: attn = q * s. The softmax denominator is
    # folded into s, so the device never divides by it elementwise. The f32
    # scale is bit-packed into the last 4 int8 columns (single output tensor
    # -> single device->host fetch).
    i8 = mybir.dt.int8
    out_d = nc.dram_tensor("attnq", [N, C + 4], i8, kind="ExternalOutput")

    with TileContext(nc) as tc:
        with (
            tc.tile_pool(name="const", bufs=1) as cst,
            tc.tile_pool(name="xp", bufs=1) as xp,
            tc.tile_pool(name="qks", bufs=1) as qks,
            tc.tile_pool(name="vtp", bufs=1) as vtp,
            tc.tile_pool(name="ep", bufs=16) as ep,
            tc.tile_pool(name="aop", bufs=8) as aop,
            tc.tile_pool(name="rcp", bufs=8) as rcp,
            tc.tile_pool(name="ps_s", bufs=3, space="PSUM") as ps_s,
            tc.tile_pool(name="ps_av", bufs=4, space="PSUM") as ps_av,
        ):
            # ---- weights / biases ----
            wqk_t = [cst.tile([128, 2 * DQK], f16, tag=f"wqk{c}", name=f"wqk{c}")
                     for c in range(2)]
            wv_t = [cst.tile([128, C], f16, tag=f"wv{c}", name=f"wv{c}")
                    for c in range(2)]
            bq_t = cst.tile([DQK, 1], f32, tag="bq", name="bq")
            bk_t = cst.tile([DQK, 1], f32, tag="bk", name="bk")
            bq16 = cst.tile([DQK, 1], f16, tag="bq16", name="bq16")
            bk16 = cst.tile([DQK, 1], f16, tag="bk16", name="bk16")
            for c in range(2):
                nc.sync.dma_start(out=wqk_t[c], in_=wqk_d[c * 128:(c + 1) * 128, :])
                nc.sync.dma_start(out=wv_t[c], in_=wv_d[c * 128:(c + 1) * 128, :])
            nc.sync.dma_start(out=bq16, in_=bqk_d[0:DQK, :])
            nc.sync.dma_start(out=bk16, in_=bqk_d[DQK:2 * DQK, :])
            nc.scalar.activation(bq_t, bq16, AF.Copy)
            nc.scalar.activation(bk_t, bk16, AF.Copy)
            ones_bt = cst.tile([128, 1], bf16, tag="ones", name="ones_bt")
            nc.gpsimd.memset(ones_bt, 1.0)

            # ---- x: 2 c-chunks x 8 n-blocks of [128, 512], fp16 ----
            x_t = [[xp.tile([128, 512], f16, tag=f"x{c}_{nb}", name=f"x{c}_{nb}")
                    for nb in range(NBLK)] for c in range(2)]
            for nb in range(NBLK):
                for c in range(2):
                    nc.sync.dma_start(
                        out=x_t[c][nb],
                        in_=x_d[c * 128:(c + 1) * 128, nb * 512:(nb + 1) * 512])

            q_sb = qks.tile([DQK, N], f16, tag="q", name="q_sb")
            k_sb = qks.tile([DQK, N], f16, tag="k", name="k_sb")

            # ---- QK projection: q = Wq @ x, k = Wk @ x  (K=C contraction) ----
            for nb in range(NBLK):
                q_ps = ps_s.tile([DQK, 512], f32, tag="s", name=f"qps{nb}")
                nc.tensor.matmul(q_ps, lhsT=wqk_t[0][:, 0:DQK],
                                 rhs=x_t[0][nb], start=True, stop=False)
                nc.tensor.matmul(q_ps, lhsT=wqk_t[1][:, 0:DQK],
                                 rhs=x_t[1][nb], start=False, stop=True)
                nc.scalar.activation(q_sb[:, nb * 512:(nb + 1) * 512], q_ps,
                                     AF.Identity, bias=bq_t)
                k_ps = ps_s.tile([DQK, 512], f32, tag="s", name=f"kps{nb}")
                nc.tensor.matmul(k_ps, lhsT=wqk_t[0][:, DQK:2 * DQK],
                                 rhs=x_t[0][nb], start=True, stop=False)
                nc.tensor.matmul(k_ps, lhsT=wqk_t[1][:, DQK:2 * DQK],
                                 rhs=x_t[1][nb], start=False, stop=True)
                nc.scalar.activation(k_sb[:, nb * 512:(nb + 1) * 512], k_ps,
                                     AF.Identity, bias=bk_t)

            # ---- V projection, directly transposed: vT[j, c] = x[:, j].T @ WvT
            # vT tiles [128 (j), 257]; col 256 = 1.0 so the AV matmul also
            # produces sum_j(E) ("ones trick") for the softmax denominator.
            vt_t = []
            for j in range(NJ):
                nb, off = divmod(j * 128, 512)
                pv = ps_av.tile([128, C], f32, tag="av", name=f"vps{j}")
                nc.tensor.matmul(pv, lhsT=x_t[0][nb][:, off:off + 128],
                                 rhs=wv_t[0], start=True, stop=False)
                nc.tensor.matmul(pv, lhsT=x_t[1][nb][:, off:off + 128],
                                 rhs=wv_t[1], start=False, stop=True)
                vt = vtp.tile([128, C + 1], bf16, tag=f"vt{j}", name=f"vt{j}")
                nc.scalar.activation(vt[:, 0:C], pv, AF.Copy)
                nc.scalar.activation(vt[:, C:C + 1], ones_bt, AF.Copy)
                vt_t.append(vt)

            # ---- attention over 8 query blocks of 512 ----
            for ib in range(NBLK):
                av_ps = [ps_av.tile([128, C + 1], f32, tag="av", name=f"av{ib}_{q}")
                         for q in range(4)]
                e_t = {}
                for j in range(NJ):
                    s_ps = ps_s.tile([128, 512], f32, tag="s", name=f"sps{ib}_{j}")
                    nc.tensor.matmul(
                        s_ps, lhsT=k_sb[:, j * 128:(j + 1) * 128],
                        rhs=q_sb[:, ib * 512:(ib + 1) * 512],
                        start=True, stop=True)
                    et = ep.tile([128, 512], bf16, tag="e", name=f"e{ib}_{j}")
                    nc.scalar.activation(et, s_ps, AF.Exp)
                    e_t[j] = et
                    if j >= 2:
                        jj = j - 2
                        for q in range(4):
                            nc.tensor.matmul(
                                av_ps[q], lhsT=e_t[jj][:, q * 128:(q + 1) * 128],
                                rhs=vt_t[jj], start=(jj == 0), stop=False)
                for jj in (NJ - 2, NJ - 1):
                    for q in range(4):
                        nc.tensor.matmul(
                            av_ps[q], lhsT=e_t[jj][:, q * 128:(q + 1) * 128],
                            rhs=vt_t[jj], start=False, stop=(jj == NJ - 1))

                # int8 quantization: q = av * (127/maxabs(av)); the softmax
                # 1/sumexp and the maxabs/127 dequant step both land in the
                # per-token scale s = maxabs * (1/sumexp) / 127.
                for q in range(4):
                    mx = rcp.tile([128, 1], f32, tag="mx", name=f"mx{ib}_{q}")
                    nc.vector.tensor_reduce(
                        mx, av_ps[q][:, 0:C], axis=mybir.AxisListType.X,
                        op=ALU.max, apply_absolute_value=True)
                    mxc = rcp.tile([128, 1], f32, tag="mxc", name=f"mxc{ib}_{q}")
                    nc.vector.tensor_scalar(mxc, mx, 1e-30, None, ALU.max)
                    rq = rcp.tile([128, 1], f32, tag="rq", name=f"rq{ib}_{q}")
                    nc.vector.reciprocal(rq, mxc)
                    ao = aop.tile([128, C], i8, tag="ao", name=f"ao{ib}_{q}")
                    nc.vector.tensor_scalar(ao, av_ps[q][:, 0:C], rq, 127.0,
                                            ALU.mult, ALU.mult)
                    r0 = ib * 512 + q * 128
                    nc.sync.dma_start(out=out_d[r0:r0 + 128, 0:C], in_=ao)
                    rs = rcp.tile([128, 1], f32, tag="rs", name=f"rs{ib}_{q}")
                    nc.vector.reciprocal(rs, av_ps[q][:, C:C + 1])
                    st = rcp.tile([128, 1], f32, tag="st", name=f"st{ib}_{q}")
                    nc.vector.tensor_scalar(st, mxc, rs, 1.0 / 127.0,
                                            ALU.mult, ALU.mult)
                    nc.sync.dma_start(out=out_d[r0:r0 + 128, C:C + 4],
                                      in_=st.bitcast(i8))

    nc.finalize()
    return nc


def _run(in_maps, trace=False):
    from concourse.bass_utils import run_bass_kernel_spmd

    if "nc" not in _CACHE:
        _CACHE["nc"] = _build_nc()
    return run_bass_kernel_spmd(
        _CACHE["nc"], in_maps, list(range(FB)),
        trace=trace, trace_cores=[0] if trace else None)


def _prep_inputs(features, Wq, bq, Wk, bk, Wv, bv, gamma):
    x_all = np.asarray(features, dtype=np.float32).reshape(FB, C, N)
    wqkT = np.concatenate([np.asarray(Wq), np.asarray(Wk)], axis=0).T  # [C, 64]
    wvT = np.asarray(Wv).T                                             # [C, C]
    bqk_col = np.zeros((C, 1), np.float32)
    bqk_col[0:DQK, 0] = np.asarray(bq, dtype=np.float32)
    bqk_col[DQK:2 * DQK, 0] = np.asarray(bk, dtype=np.float32)
    pack = np.empty((FB, C, N + 2 * DQK + C + 1), np.float16)
    pack[:, :, 0:N] = x_all
    pack[:, :, N:N + 2 * DQK] = wqkT.astype(np.float16)
    pack[:, :, N + 2 * DQK:N + 2 * DQK + C] = wvT.astype(np.float16)
    pack[:, :, N + 2 * DQK + C:] = bqk_col.astype(np.float16)
    return [{"pack": pack[i]} for i in range(FB)]


def kernel(features, Wq, bq, Wk, bk, Wv, bv, gamma):
    in_maps = _prep_inputs(features, Wq, bq, Wk, bk, Wv, bv, gamma)
    res = _run(in_maps, trace=False)
    # device returns int8 attention with per-token fp32 scales bit-packed in
    # the last 4 columns; dequant + the epilogue gamma * (attn + bv) + x run
    # here in fp32.
    raw = np.stack([res.results[i]["attnq"] for i in range(FB)], axis=0)
    scales = np.ascontiguousarray(raw[:, :, C:C + 4]).view(np.float32)
    attn = raw[:, :, 0:C].astype(np.float32) * scales        # [FB, N, C]
    attn = attn.transpose(0, 2, 1)                           # [FB, C, N]
    x_all = np.asarray(features, dtype=np.float32).reshape(FB, C, N)
    g = np.float32(np.asarray(gamma, dtype=np.float32).reshape(-1)[0])
    bvv = np.asarray(bv, dtype=np.float32).reshape(1, C, 1)
    out = g * (attn + bvv) + x_all
    return out.reshape(F, B, C, HH, WW).astype(np.float32)



# revision 2
# speedup vs baseline: 1.5634x; 1.5634x over previous
"""CrossFrameAttention Trainium2 kernel.

Full (unsharded) inputs -> full output. Internally: data-parallel over the
fused frame*batch dim (F*B = 8 elements, one per NeuronCore), weights
replicated. Per core, a fused 1x1-conv QKV projection + softmax attention
written in Bass/Tile.

The warm wall-clock of a call is dominated by the axon tunnel (~70-90 MB/s
serial pipe with ~50 ms fixed upload cost and ~100 ms fixed fetch cost), so
the kernel is shaped around minimizing tunnel bytes and per-call overhead:
  - x ships as int8 with one fp32 scale per (core, channel) row, bit-packed
    into the same tensor (8.4 MB up instead of 16.8 MB fp16); the device
    upcasts to fp16 with the per-partition scale before the matmuls;
  - projection weights ship in a separate small fp16 tensor that stays
    device-resident across calls (re-uploaded only if the weight bytes
    change);
  - the donated output buffer is the previous call's device-side output
    (the kernel writes every output byte), so no zero-buffer upload;
  - the jitted SPMD callable is built once and cached (no per-call
    re-trace), and the JAX persistent compilation cache makes the first
    call's XLA+NEFF compile a disk hit;
  - the attention output returns as int8 with per-token fp32 scales (the
    softmax denominator and the int8 dequant step are folded into one
    scale); the cheap epilogue gamma * (attn + bv) + x runs on the host
    in fp32.

Self-contained: hardcodes shapes from the problem spec.
"""

import numpy as np

import jax

# First-call XLA+NEFF compiles become disk hits across processes.
jax.config.update("jax_compilation_cache_dir", "/tmp/jax_bass_cc")
jax.config.update("jax_persistent_cache_min_compile_time_secs", 0.0)
jax.config.update("jax_persistent_cache_min_entry_size_bytes", -1)

F, B, C, HH, WW = 4, 2, 256, 64, 64
N = HH * WW            # 4096 tokens per (frame,batch) element
FB = F * B             # 8 == n_cores
DQK = 32               # q/k channel dim (C/8)
NBLK = N // 512        # 8 query blocks of 512
NJ = N // 128          # 32 key chunks of 128
PACKX = N + 4          # int8 x row + bit-packed fp32 scale
WCOLS = 2 * DQK + C + 1  # [WqT | WkT | WvT | bqk] columns, fp16

_CACHE = {}


def _build_nc():
    import concourse.mybir as mybir
    from concourse import bacc
    from concourse.tile import TileContext

    f32 = mybir.dt.float32
    f16 = mybir.dt.float16
    bf16 = mybir.dt.bfloat16
    i8 = mybir.dt.int8
    AF = mybir.ActivationFunctionType
    ALU = mybir.AluOpType

    nc = bacc.Bacc(None, target_bir_lowering=False, debug=False)

    # x int8 [C, N] with the per-channel fp32 dequant scale bit-packed into
    # the last 4 int8 columns (single per-call upload tensor).
    x_d = nc.dram_tensor("xq", [C, PACKX], i8, kind="ExternalInput")
    # weights fp16, device-resident across calls: [WqT | WkT | WvT | bqk]
    w_d = nc.dram_tensor("wts", [C, WCOLS], f16, kind="ExternalInput")
    wqk_d = w_d[:, 0:2 * DQK]
    wv_d = w_d[:, 2 * DQK:2 * DQK + C]
    bqk_d = w_d[:, WCOLS - 1:WCOLS]
    # attention output, [token, channel] layout, int8 with one fp32 scale
    # per token: attn = q * s. The softmax denominator is folded into s, so
    # the device never divides by it elementwise. The f32 scale is
    # bit-packed into the last 4 int8 columns.
    out_d = nc.dram_tensor("attnq", [N, C + 4], i8, kind="ExternalOutput")

    with TileContext(nc) as tc:
        with (
            tc.tile_pool(name="const", bufs=1) as cst,
            tc.tile_pool(name="x8p", bufs=1) as x8p,
            tc.tile_pool(name="xp", bufs=1) as xp,
            tc.tile_pool(name="qks", bufs=1) as qks,
            tc.tile_pool(name="vtp", bufs=1) as vtp,
            tc.tile_pool(name="ep", bufs=16) as ep,
            tc.tile_pool(name="aop", bufs=8) as aop,
            tc.tile_pool(name="rcp", bufs=8) as rcp,
            tc.tile_pool(name="ps_s", bufs=3, space="PSUM") as ps_s,
            tc.tile_pool(name="ps_av", bufs=4, space="PSUM") as ps_av,
        ):
            # ---- weights / biases ----
            wqk_t = [cst.tile([128, 2 * DQK], f16, tag=f"wqk{c}", name=f"wqk{c}")
                     for c in range(2)]
            wv_t = [cst.tile([128, C], f16, tag=f"wv{c}", name=f"wv{c}")
                    for c in range(2)]
            bq_t = cst.tile([DQK, 1], f32, tag="bq", name="bq")
            bk_t = cst.tile([DQK, 1], f32, tag="bk", name="bk")
            bq16 = cst.tile([DQK, 1], f16, tag="bq16", name="bq16")
            bk16 = cst.tile([DQK, 1], f16, tag="bk16", name="bk16")
            for c in range(2):
                nc.sync.dma_start(out=wqk_t[c], in_=wqk_d[c * 128:(c + 1) * 128, :])
                nc.sync.dma_start(out=wv_t[c], in_=wv_d[c * 128:(c + 1) * 128, :])
            nc.sync.dma_start(out=bq16, in_=bqk_d[0:DQK, :])
            nc.sync.dma_start(out=bk16, in_=bqk_d[DQK:2 * DQK, :])
            nc.scalar.activation(bq_t, bq16, AF.Copy)
            nc.scalar.activation(bk_t, bk16, AF.Copy)
            ones_bt = cst.tile([128, 1], bf16, tag="ones", name="ones_bt")
            nc.gpsimd.memset(ones_bt, 1.0)

            # ---- x: int8 tiles + per-channel scale -> fp16 tiles ----
            s8 = [cst.tile([128, 4], i8, tag=f"s8_{c}", name=f"s8_{c}")
                  for c in range(2)]
            for c in range(2):
                nc.sync.dma_start(out=s8[c],
                                  in_=x_d[c * 128:(c + 1) * 128, N:N + 4])
            sf = [s8[c].bitcast(f32) for c in range(2)]

            x8_t = [[x8p.tile([128, 512], i8, tag=f"x8_{c}_{nb}",
                              name=f"x8_{c}_{nb}") for nb in range(NBLK)]
                    for c in range(2)]
            x_t = [[xp.tile([128, 512], f16, tag=f"x{c}_{nb}", name=f"x{c}_{nb}")
                    for nb in range(NBLK)] for c in range(2)]
            for nb in range(NBLK):
                for c in range(2):
                    nc.sync.dma_start(
                        out=x8_t[c][nb],
                        in_=x_d[c * 128:(c + 1) * 128, nb * 512:(nb + 1) * 512])
                    nc.scalar.activation(x_t[c][nb], x8_t[c][nb], AF.Identity,
                                         scale=sf[c])

            q_sb = qks.tile([DQK, N], f16, tag="q", name="q_sb")
            k_sb = qks.tile([DQK, N], f16, tag="k", name="k_sb")

            # ---- QK projection: q = Wq @ x, k = Wk @ x  (K=C contraction) ----
            for nb in range(NBLK):
                q_ps = ps_s.tile([DQK, 512], f32, tag="s", name=f"qps{nb}")
                nc.tensor.matmul(q_ps, lhsT=wqk_t[0][:, 0:DQK],
                                 rhs=x_t[0][nb], start=True, stop=False)
                nc.tensor.matmul(q_ps, lhsT=wqk_t[1][:, 0:DQK],
                                 rhs=x_t[1][nb], start=False, stop=True)
                nc.scalar.activation(q_sb[:, nb * 512:(nb + 1) * 512], q_ps,
                                     AF.Identity, bias=bq_t)
                k_ps = ps_s.tile([DQK, 512], f32, tag="s", name=f"kps{nb}")
                nc.tensor.matmul(k_ps, lhsT=wqk_t[0][:, DQK:2 * DQK],
                                 rhs=x_t[0][nb], start=True, stop=False)
                nc.tensor.matmul(k_ps, lhsT=wqk_t[1][:, DQK:2 * DQK],
                                 rhs=x_t[1][nb], start=False, stop=True)
                nc.scalar.activation(k_sb[:, nb * 512:(nb + 1) * 512], k_ps,
                                     AF.Identity, bias=bk_t)

            # ---- V projection, directly transposed: vT[j, c] = x[:, j].T @ WvT
            # vT tiles [128 (j), 257]; col 256 = 1.0 so the AV matmul also
            # produces sum_j(E) ("ones trick") for the softmax denominator.
            vt_t = []
            for j in range(NJ):
                nb, off = divmod(j * 128, 512)
                pv = ps_av.tile([128, C], f32, tag="av", name=f"vps{j}")
                nc.tensor.matmul(pv, lhsT=x_t[0][nb][:, off:off + 128],
                                 rhs=wv_t[0], start=True, stop=False)
                nc.tensor.matmul(pv, lhsT=x_t[1][nb][:, off:off + 128],
                                 rhs=wv_t[1], start=False, stop=True)
                vt = vtp.tile([128, C + 1], bf16, tag=f"vt{j}", name=f"vt{j}")
                nc.scalar.activation(vt[:, 0:C], pv, AF.Copy)
                nc.scalar.activation(vt[:, C:C + 1], ones_bt, AF.Copy)
                vt_t.append(vt)

            # ---- attention over 8 query blocks of 512 ----
            for ib in range(NBLK):
                av_ps = [ps_av.tile([128, C + 1], f32, tag="av", name=f"av{ib}_{q}")
                         for q in range(4)]
                e_t = {}
                for j in range(NJ):
                    s_ps = ps_s.tile([128, 512], f32, tag="s", name=f"sps{ib}_{j}")
                    nc.tensor.matmul(
                        s_ps, lhsT=k_sb[:, j * 128:(j + 1) * 128],
                        rhs=q_sb[:, ib * 512:(ib + 1) * 512],
                        start=True, stop=True)
                    et = ep.tile([128, 512], bf16, tag="e", name=f"e{ib}_{j}")
                    nc.scalar.activation(et, s_ps, AF.Exp)
                    e_t[j] = et
                    if j >= 2:
                        jj = j - 2
                        for q in range(4):
                            nc.tensor.matmul(
                                av_ps[q], lhsT=e_t[jj][:, q * 128:(q + 1) * 128],
                                rhs=vt_t[jj], start=(jj == 0), stop=False)
                for jj in (NJ - 2, NJ - 1):
                    for q in range(4):
                        nc.tensor.matmul(
                            av_ps[q], lhsT=e_t[jj][:, q * 128:(q + 1) * 128],
                            rhs=vt_t[jj], start=False, stop=(jj == NJ - 1))

                # int8 quantization: q = av * (127/maxabs(av)); the softmax
                # 1/sumexp and the maxabs/127 dequant step both land in the
                # per-token scale s = maxabs * (1/sumexp) / 127.
                for q in range(4):
                    mx = rcp.tile([128, 1], f32, tag="mx", name=f"mx{ib}_{q}")
                    nc.vector.tensor_reduce(
                        mx, av_ps[q][:, 0:C], axis=mybir.AxisListType.X,
                        op=ALU.max, apply_absolute_value=True)
                    mxc = rcp.tile([128, 1], f32, tag="mxc", name=f"mxc{ib}_{q}")
                    nc.vector.tensor_scalar(mxc, mx, 1e-30, None, ALU.max)
                    rq = rcp.tile([128, 1], f32, tag="rq", name=f"rq{ib}_{q}")
                    nc.vector.reciprocal(rq, mxc)
                    ao = aop.tile([128, C], i8, tag="ao", name=f"ao{ib}_{q}")
                    nc.vector.tensor_scalar(ao, av_ps[q][:, 0:C], rq, 127.0,
                                            ALU.mult, ALU.mult)
                    r0 = ib * 512 + q * 128
                    nc.sync.dma_start(out=out_d[r0:r0 + 128, 0:C], in_=ao)
                    rs = rcp.tile([128, 1], f32, tag="rs", name=f"rs{ib}_{q}")
                    nc.vector.reciprocal(rs, av_ps[q][:, C:C + 1])
                    st = rcp.tile([128, 1], f32, tag="st", name=f"st{ib}_{q}")
                    nc.vector.tensor_scalar(st, mxc, rs, 1.0 / 127.0,
                                            ALU.mult, ALU.mult)
                    nc.sync.dma_start(out=out_d[r0:r0 + 128, C:C + 4],
                                      in_=st.bitcast(i8))

    nc.finalize()
    return nc


def _get_nc():
    if "nc" not in _CACHE:
        _CACHE["nc"] = _build_nc()
    return _CACHE["nc"]


def _build_fast():
    """One-time setup of the cached jitted SPMD callable (replicates
    bass_utils.run_bass_kernel_spmd's axon path, hoisted out of the
    per-call loop so warm calls skip the re-trace)."""
    import jax.numpy as jnp
    from jax.sharding import Mesh, PartitionSpec, NamedSharding
    from jax.experimental.shard_map import shard_map
    from concourse import bass2jax
    from concourse.bass2jax import _bass_exec_p, install_neuronx_cc_hook
    import concourse.mybir as mybir

    install_neuronx_cc_hook()
    nc = _get_nc()

    partition_name = (nc.partition_id_tensor.name
                      if nc.partition_id_tensor else None)
    in_names, out_names, out_avals = [], [], []
    for alloc in nc.m.functions[0].allocations:
        if not isinstance(alloc, mybir.MemoryLocationSet):
            continue
        name = alloc.memorylocations[0].name
        if alloc.kind == "ExternalInput":
            if name != partition_name:
                in_names.append(name)
        elif alloc.kind == "ExternalOutput":
            out_names.append(name)
            out_avals.append(jax.core.ShapedArray(
                tuple(alloc.tensor_shape), mybir.dt.np(alloc.dtype)))
    assert in_names == ["xq", "wts"] and out_names == ["attnq"], (
        in_names, out_names)
    n_params = len(in_names)
    all_in = in_names + out_names
    if partition_name is not None:
        all_in.append(partition_name)
    donate = tuple(range(n_params, n_params + len(out_names)))

    def _body(*args):
        operands = list(args)
        if partition_name is not None:
            operands.append(bass2jax.partition_id_tensor())
        outs = _bass_exec_p.bind(
            *operands, out_avals=tuple(out_avals), in_names=tuple(all_in),
            out_names=tuple(out_names), lowering_input_output_aliases=(),
            sim_require_finite=True, sim_require_nnan=True, nc=nc)
        return tuple(outs)

    devices = jax.devices()[:FB]
    mesh = Mesh(np.asarray(devices), ("core",))
    n_args = n_params + len(out_names)
    jitf = jax.jit(
        shard_map(_body, mesh=mesh,
                  in_specs=(PartitionSpec("core"),) * n_args,
                  out_specs=(PartitionSpec("core"),) * len(out_names),
                  check_rep=False),
        donate_argnums=donate, keep_unused=True)
    sh = NamedSharding(mesh, PartitionSpec("core"))
    zeros_fn = jax.jit(lambda: jnp.zeros((FB * N, C + 4), jnp.int8),
                       out_shardings=sh)
    _CACHE["jitf"] = jitf
    _CACHE["sharding"] = sh
    _CACHE["zeros_fn"] = zeros_fn


class _Res:
    __slots__ = ("out", "results", "exec_time_ns", "mean_exec_time_ns",
                 "instructions_and_trace", "profile_json")

    def __init__(self, out):
        self.out = out                                   # [FB, N, C+4] int8
        self.results = [{"attnq": out[i]} for i in range(FB)]
        self.exec_time_ns = None
        self.mean_exec_time_ns = None
        self.instructions_and_trace = None
        self.profile_json = None


def _per_core_maps(in_):
    xq, wts = in_["xq"], in_["wts"]
    return [{"xq": xq[i * C:(i + 1) * C], "wts": wts[i * C:(i + 1) * C]}
            for i in range(FB)]


def _run(in_, trace=False):
    if trace or "spmd_done" not in _CACHE:
        # compile + run via bass_utils (first call / trace requests); warms
        # the persistent cache with the identical jit the fast path reuses.
        from concourse.bass_utils import run_bass_kernel_spmd
        res = run_bass_kernel_spmd(
            _get_nc(), _per_core_maps(in_), list(range(FB)),
            trace=trace, trace_cores=[0] if trace else None)
        _CACHE["spmd_done"] = True
        out = np.stack([res.results[i]["attnq"] for i in range(FB)], axis=0)
        r = _Res(out)
        r.exec_time_ns = res.exec_time_ns
        r.mean_exec_time_ns = getattr(res, "mean_exec_time_ns", None)
        r.instructions_and_trace = res.instructions_and_trace
        return r

    if "jitf" not in _CACHE:
        _build_fast()
    wts_dev = _CACHE.get("wts_dev")
    if wts_dev is None or not _CACHE.get("wts_dev_ok", False):
        wts_dev = jax.device_put(in_["wts"], _CACHE["sharding"])
        _CACHE["wts_dev"] = wts_dev
        _CACHE["wts_dev_ok"] = True
    donate_buf = _CACHE.pop("donate", None)
    if donate_buf is None:
        donate_buf = _CACHE["zeros_fn"]()
    outs = _CACHE["jitf"](in_["xq"], wts_dev, donate_buf)
    flat = np.asarray(outs[0])                           # fetch
    _CACHE["donate"] = outs[0]
    return _Res(flat.reshape(FB, N, C + 4))


def _weights_np(Wq, bq, Wk, bk, Wv):
    """Build (and cache) the replicated fp16 weight tensor [FB*C, WCOLS]."""
    wc = _CACHE.get("wts_src")
    args = (Wq, bq, Wk, bk, Wv)
    if wc is not None and all(
            np.array_equal(a, b) for a, b in zip(wc, args)):
        return _CACHE["wts_np"]
    w = np.empty((C, WCOLS), np.float16)
    w[:, 0:DQK] = np.asarray(Wq, np.float32).T
    w[:, DQK:2 * DQK] = np.asarray(Wk, np.float32).T
    w[:, 2 * DQK:2 * DQK + C] = np.asarray(Wv, np.float32).T
    col = np.zeros((C,), np.float32)
    col[0:DQK] = np.asarray(bq, np.float32)
    col[DQK:2 * DQK] = np.asarray(bk, np.float32)
    w[:, WCOLS - 1] = col
    wts = np.tile(w, (FB, 1))
    _CACHE["wts_src"] = tuple(np.array(a, copy=True) for a in args)
    _CACHE["wts_np"] = wts
    _CACHE["wts_dev_ok"] = False                          # force re-upload
    return wts


def _prep_inputs(features, Wq, bq, Wk, bk, Wv, bv, gamma):
    x_all = np.asarray(features, dtype=np.float32).reshape(FB * C, N)
    buf = _CACHE.get("xq_buf")
    tmp = _CACHE.get("tmp_f32")
    if buf is None:
        buf = np.empty((FB * C, PACKX), np.int8)
        tmp = np.empty((FB * C, N), np.float32)
        _CACHE["xq_buf"] = buf
        _CACHE["tmp_f32"] = tmp
    # per-(core,channel) symmetric int8 quantization
    np.abs(x_all, out=tmp)
    amax = tmp.max(axis=1)
    np.maximum(amax, 1e-20, out=amax)
    inv = np.float32(127.0) / amax
    np.multiply(x_all, inv[:, None], out=tmp)
    np.rint(tmp, out=tmp)
    np.copyto(buf[:, 0:N], tmp, casting="unsafe")
    scale = (amax * np.float32(1.0 / 127.0)).astype(np.float32)
    buf[:, N:N + 4] = scale.reshape(-1, 1).view(np.int8)
    wts = _weights_np(Wq, bq, Wk, bk, Wv)
    return {"xq": buf, "wts": wts}


def kernel(features, Wq, bq, Wk, bk, Wv, bv, gamma):
    in_ = _prep_inputs(features, Wq, bq, Wk, bk, Wv, bv, gamma)
    res = _run(in_, trace=False)
    # device returns int8 attention with per-token fp32 scales bit-packed in
    # the last 4 columns; dequant + the epilogue gamma * (attn + bv) + x run
    # here in fp32.
    raw = res.out                                        # [FB, N, C+4] int8
    scales = np.ascontiguousarray(raw[:, :, C:C + 4]).view(np.float32)
    attn = raw[:, :, 0:C].astype(np.float32) * scales    # [FB, N, C]
    attn = attn.transpose(0, 2, 1)                       # [FB, C, N]
    x_all = np.asarray(features, dtype=np.float32).reshape(FB, C, N)
    g = np.float32(np.asarray(gamma, dtype=np.float32).reshape(-1)[0])
    bvv = np.asarray(bv, dtype=np.float32).reshape(1, C, 1)
    out = g * (attn + bvv) + x_all
    return out.reshape(F, B, C, HH, WW).astype(np.float32)


# revision 11
# speedup vs baseline: 1.5951x; 1.0203x over previous
"""CrossFrameAttention Trainium2 kernel.

Full (unsharded) inputs -> full output. Internally: data-parallel over the
fused frame*batch dim (F*B = 8 elements, one per NeuronCore), weights
replicated. Per core, a fused 1x1-conv QKV projection + softmax attention
written in Bass/Tile.

The warm wall-clock of a call is dominated by the axon tunnel (~70-90 MB/s
serial pipe with ~50 ms fixed upload cost and ~100 ms fixed fetch cost), so
the kernel is shaped around minimizing tunnel bytes and per-call overhead:
  - x ships as int8 with one fp32 scale per (core, channel) row, bit-packed
    into the same tensor (8.4 MB up instead of 16.8 MB fp16); the device
    upcasts to fp16 with the per-partition scale before the matmuls;
  - int8 noise on x would be amplified by the softmax (logits have std ~6,
    so 0.9% q/k noise moves dominant attention weights by ~7%), so the host
    also ships an int8 correction dq = Wqk @ (x - dequant(x_int8)) + b
    (+0.26 MB/core) that the device adds to its coarse q/k — q/k reach
    ~fp16 precision while x stays int8 for the V path, where the noise is
    benign;
  - projection weights ship in a separate small fp16 tensor that stays
    device-resident across calls (re-uploaded only if the weight bytes
    change);
  - the donated output buffer is the previous call's device-side output
    (the kernel writes every output byte), so no zero-buffer upload;
  - the jitted SPMD callable is built once and cached (no per-call
    re-trace), and the JAX persistent compilation cache makes the first
    call's XLA+NEFF compile a disk hit;
  - the attention output returns as int8 with per-token fp32 scales (the
    softmax denominator and the int8 dequant step are folded into one
    scale); the cheap epilogue gamma * (attn + bv) + x runs on the host
    in fp32.

Self-contained: hardcodes shapes from the problem spec.
"""

import numpy as np

import jax

# First-call XLA+NEFF compiles become disk hits across processes.
jax.config.update("jax_compilation_cache_dir", "/tmp/jax_bass_cc")
jax.config.update("jax_persistent_cache_min_compile_time_secs", 0.0)
jax.config.update("jax_persistent_cache_min_entry_size_bytes", -1)

F, B, C, HH, WW = 4, 2, 256, 64, 64
N = HH * WW            # 4096 tokens per (frame,batch) element
FB = F * B             # 8 == n_cores
DQK = 32               # q/k channel dim (C/8)
NBLK = N // 512        # 8 query blocks of 512
NJ = N // 128          # 32 key chunks of 128
PACKX = N + 4          # int8 x row + bit-packed fp32 scale
WCOLS = 2 * DQK + C + 1  # [WqT | WkT | WvT | bqk] columns, fp16

_CACHE = {}


def _build_nc():
    import concourse.mybir as mybir
    from concourse import bacc
    from concourse.tile import TileContext

    f32 = mybir.dt.float32
    f16 = mybir.dt.float16
    bf16 = mybir.dt.bfloat16
    i8 = mybir.dt.int8
    AF = mybir.ActivationFunctionType
    ALU = mybir.AluOpType

    nc = bacc.Bacc(None, target_bir_lowering=False, debug=False)

    # x int8 [C, N] with the per-channel fp32 dequant scale bit-packed into
    # the last 4 int8 columns (single per-call upload tensor).
    x_d = nc.dram_tensor("xq", [C, PACKX], i8, kind="ExternalInput")
    # q/k correction, int8 rows [q | k] with per-row fp32 scales bit-packed
    # into the last 4 columns; includes the bq/bk biases.
    dq_d = nc.dram_tensor("dqk", [2 * DQK, PACKX], i8, kind="ExternalInput")
    # weights fp16, device-resident across calls: [WqT | WkT | WvT | bqk]
    w_d = nc.dram_tensor("wts", [C, WCOLS], f16, kind="ExternalInput")
    wqk_d = w_d[:, 0:2 * DQK]
    wv_d = w_d[:, 2 * DQK:2 * DQK + C]
    # attention output, [token, channel] layout, int8 with one fp32 scale
    # per token: attn = q * s. The softmax denominator is folded into s, so
    # the device never divides by it elementwise. The f32 scale is
    # bit-packed into the last 4 int8 columns.
    out_d = nc.dram_tensor("attnq", [N, C + 4], i8, kind="ExternalOutput")

    with TileContext(nc) as tc:
        with (
            tc.tile_pool(name="const", bufs=1) as cst,
            tc.tile_pool(name="x8p", bufs=1) as x8p,
            tc.tile_pool(name="xp", bufs=1) as xp,
            tc.tile_pool(name="qks", bufs=1) as qks,
            tc.tile_pool(name="vtp", bufs=1) as vtp,
            tc.tile_pool(name="ep", bufs=16) as ep,
            tc.tile_pool(name="aop", bufs=8) as aop,
            tc.tile_pool(name="rcp", bufs=8) as rcp,
            tc.tile_pool(name="ps_s", bufs=3, space="PSUM") as ps_s,
            tc.tile_pool(name="ps_av", bufs=4, space="PSUM") as ps_av,
        ):
            # ---- weights / biases ----
            wqk_t = [cst.tile([128, 2 * DQK], f16, tag=f"wqk{c}", name=f"wqk{c}")
                     for c in range(2)]
            wv_t = [cst.tile([128, C], f16, tag=f"wv{c}", name=f"wv{c}")
                    for c in range(2)]
            for c in range(2):
                nc.sync.dma_start(out=wqk_t[c], in_=wqk_d[c * 128:(c + 1) * 128, :])
                nc.sync.dma_start(out=wv_t[c], in_=wv_d[c * 128:(c + 1) * 128, :])
            ones_bt = cst.tile([128, 1], bf16, tag="ones", name="ones_bt")
            nc.gpsimd.memset(ones_bt, 1.0)

            # ---- q/k int8 correction rows -> fp16 [DQK, N] tiles ----
            dq8 = [cst.tile([DQK, N], i8, tag=f"dq8_{h}", name=f"dq8_{h}")
                   for h in range(2)]
            ds8 = [cst.tile([DQK, 4], i8, tag=f"ds8_{h}", name=f"ds8_{h}")
                   for h in range(2)]
            dqf = [cst.tile([DQK, N], f16, tag=f"dqf_{h}", name=f"dqf_{h}")
                   for h in range(2)]
            for h in range(2):
                nc.sync.dma_start(out=dq8[h],
                                  in_=dq_d[h * DQK:(h + 1) * DQK, 0:N])
                nc.sync.dma_start(out=ds8[h],
                                  in_=dq_d[h * DQK:(h + 1) * DQK, N:N + 4])
                nc.scalar.activation(dqf[h], dq8[h], AF.Identity,
                                     scale=ds8[h].bitcast(f32))

            # ---- x: int8 tiles + per-channel scale -> fp16 tiles ----
            s8 = [cst.tile([128, 4], i8, tag=f"s8_{c}", name=f"s8_{c}")
                  for c in range(2)]
            for c in range(2):
                nc.sync.dma_start(out=s8[c],
                                  in_=x_d[c * 128:(c + 1) * 128, N:N + 4])
            sf = [s8[c].bitcast(f32) for c in range(2)]

            x8_t = [[x8p.tile([128, 512], i8, tag=f"x8_{c}_{nb}",
                              name=f"x8_{c}_{nb}") for nb in range(NBLK)]
                    for c in range(2)]
            x_t = [[xp.tile([128, 512], f16, tag=f"x{c}_{nb}", name=f"x{c}_{nb}")
                    for nb in range(NBLK)] for c in range(2)]
            for nb in range(NBLK):
                for c in range(2):
                    nc.sync.dma_start(
                        out=x8_t[c][nb],
                        in_=x_d[c * 128:(c + 1) * 128, nb * 512:(nb + 1) * 512])
                    nc.scalar.activation(x_t[c][nb], x8_t[c][nb], AF.Identity,
                                         scale=sf[c])

            q_sb = qks.tile([DQK, N], f16, tag="q", name="q_sb")
            k_sb = qks.tile([DQK, N], f16, tag="k", name="k_sb")

            # ---- QK projection: q = Wq @ x + dq, k = Wk @ x + dk ----
            # (bias is folded into the host-computed dq/dk correction)
            for nb in range(NBLK):
                blk = slice(nb * 512, (nb + 1) * 512)
                q_ps = ps_s.tile([DQK, 512], f32, tag="s", name=f"qps{nb}")
                nc.tensor.matmul(q_ps, lhsT=wqk_t[0][:, 0:DQK],
                                 rhs=x_t[0][nb], start=True, stop=False)
                nc.tensor.matmul(q_ps, lhsT=wqk_t[1][:, 0:DQK],
                                 rhs=x_t[1][nb], start=False, stop=True)
                nc.vector.scalar_tensor_tensor(
                    q_sb[:, blk], q_ps, 1.0, dqf[0][:, blk],
                    ALU.mult, ALU.add)
                k_ps = ps_s.tile([DQK, 512], f32, tag="s", name=f"kps{nb}")
                nc.tensor.matmul(k_ps, lhsT=wqk_t[0][:, DQK:2 * DQK],
                                 rhs=x_t[0][nb], start=True, stop=False)
                nc.tensor.matmul(k_ps, lhsT=wqk_t[1][:, DQK:2 * DQK],
                                 rhs=x_t[1][nb], start=False, stop=True)
                nc.vector.scalar_tensor_tensor(
                    k_sb[:, blk], k_ps, 1.0, dqf[1][:, blk],
                    ALU.mult, ALU.add)

            # ---- V projection, directly transposed: vT[j, c] = x[:, j].T @ WvT
            # vT tiles [128 (j), 257]; col 256 = 1.0 so the AV matmul also
            # produces sum_j(E) ("ones trick") for the softmax denominator.
            vt_t = []
            for j in range(NJ):
                nb, off = divmod(j * 128, 512)
                pv = ps_av.tile([128, C], f32, tag="av", name=f"vps{j}")
                nc.tensor.matmul(pv, lhsT=x_t[0][nb][:, off:off + 128],
                                 rhs=wv_t[0], start=True, stop=False)
                nc.tensor.matmul(pv, lhsT=x_t[1][nb][:, off:off + 128],
                                 rhs=wv_t[1], start=False, stop=True)
                vt = vtp.tile([128, C + 1], bf16, tag=f"vt{j}", name=f"vt{j}")
                nc.scalar.activation(vt[:, 0:C], pv, AF.Copy)
                nc.scalar.activation(vt[:, C:C + 1], ones_bt, AF.Copy)
                vt_t.append(vt)

            # ---- attention over 8 query blocks of 512 ----
            for ib in range(NBLK):
                av_ps = [ps_av.tile([128, C + 1], f32, tag="av", name=f"av{ib}_{q}")
                         for q in range(4)]
                e_t = {}
                for j in range(NJ):
                    s_ps = ps_s.tile([128, 512], f32, tag="s", name=f"sps{ib}_{j}")
                    nc.tensor.matmul(
                        s_ps, lhsT=k_sb[:, j * 128:(j + 1) * 128],
                        rhs=q_sb[:, ib * 512:(ib + 1) * 512],
                        start=True, stop=True)
                    et = ep.tile([128, 512], bf16, tag="e", name=f"e{ib}_{j}")
                    nc.scalar.activation(et, s_ps, AF.Exp)
                    e_t[j] = et
                    if j >= 2:
                        jj = j - 2
                        for q in range(4):
                            nc.tensor.matmul(
                                av_ps[q], lhsT=e_t[jj][:, q * 128:(q + 1) * 128],
                                rhs=vt_t[jj], start=(jj == 0), stop=False)
                for jj in (NJ - 2, NJ - 1):
                    for q in range(4):
                        nc.tensor.matmul(
                            av_ps[q], lhsT=e_t[jj][:, q * 128:(q + 1) * 128],
                            rhs=vt_t[jj], start=False, stop=(jj == NJ - 1))

                # int8 quantization: q = av * (127/maxabs(av)); the softmax
                # 1/sumexp and the maxabs/127 dequant step both land in the
                # per-token scale s = maxabs * (1/sumexp) / 127.
                for q in range(4):
                    mx = rcp.tile([128, 1], f32, tag="mx", name=f"mx{ib}_{q}")
                    nc.vector.tensor_reduce(
                        mx, av_ps[q][:, 0:C], axis=mybir.AxisListType.X,
                        op=ALU.max, apply_absolute_value=True)
                    mxc = rcp.tile([128, 1], f32, tag="mxc", name=f"mxc{ib}_{q}")
                    nc.vector.tensor_scalar(mxc, mx, 1e-30, None, ALU.max)
                    rq = rcp.tile([128, 1], f32, tag="rq", name=f"rq{ib}_{q}")
                    nc.vector.reciprocal(rq, mxc)
                    ao = aop.tile([128, C], i8, tag="ao", name=f"ao{ib}_{q}")
                    nc.vector.tensor_scalar(ao, av_ps[q][:, 0:C], rq, 127.0,
                                            ALU.mult, ALU.mult)
                    r0 = ib * 512 + q * 128
                    nc.sync.dma_start(out=out_d[r0:r0 + 128, 0:C], in_=ao)
                    rs = rcp.tile([128, 1], f32, tag="rs", name=f"rs{ib}_{q}")
                    nc.vector.reciprocal(rs, av_ps[q][:, C:C + 1])
                    st = rcp.tile([128, 1], f32, tag="st", name=f"st{ib}_{q}")
                    nc.vector.tensor_scalar(st, mxc, rs, 1.0 / 127.0,
                                            ALU.mult, ALU.mult)
                    nc.sync.dma_start(out=out_d[r0:r0 + 128, C:C + 4],
                                      in_=st.bitcast(i8))

    nc.finalize()
    return nc


def _get_nc():
    if "nc" not in _CACHE:
        _CACHE["nc"] = _build_nc()
    return _CACHE["nc"]


def _build_fast():
    """One-time setup of the cached jitted SPMD callable (replicates
    bass_utils.run_bass_kernel_spmd's axon path, hoisted out of the
    per-call loop so warm calls skip the re-trace)."""
    import jax.numpy as jnp
    from jax.sharding import Mesh, PartitionSpec, NamedSharding
    from jax.experimental.shard_map import shard_map
    from concourse import bass2jax
    from concourse.bass2jax import _bass_exec_p, install_neuronx_cc_hook
    import concourse.mybir as mybir

    install_neuronx_cc_hook()
    nc = _get_nc()

    partition_name = (nc.partition_id_tensor.name
                      if nc.partition_id_tensor else None)
    in_names, out_names, out_avals = [], [], []
    for alloc in nc.m.functions[0].allocations:
        if not isinstance(alloc, mybir.MemoryLocationSet):
            continue
        name = alloc.memorylocations[0].name
        if alloc.kind == "ExternalInput":
            if name != partition_name:
                in_names.append(name)
        elif alloc.kind == "ExternalOutput":
            out_names.append(name)
            out_avals.append(jax.core.ShapedArray(
                tuple(alloc.tensor_shape), mybir.dt.np(alloc.dtype)))
    assert in_names == ["xq", "dqk", "wts"] and out_names == ["attnq"], (
        in_names, out_names)
    n_params = len(in_names)
    all_in = in_names + out_names
    if partition_name is not None:
        all_in.append(partition_name)
    donate = tuple(range(n_params, n_params + len(out_names)))

    def _body(*args):
        operands = list(args)
        if partition_name is not None:
            operands.append(bass2jax.partition_id_tensor())
        outs = _bass_exec_p.bind(
            *operands, out_avals=tuple(out_avals), in_names=tuple(all_in),
            out_names=tuple(out_names), lowering_input_output_aliases=(),
            sim_require_finite=True, sim_require_nnan=True, nc=nc)
        return tuple(outs)

    devices = jax.devices()[:FB]
    mesh = Mesh(np.asarray(devices), ("core",))
    n_args = n_params + len(out_names)
    jitf = jax.jit(
        shard_map(_body, mesh=mesh,
                  in_specs=(PartitionSpec("core"),) * n_args,
                  out_specs=(PartitionSpec("core"),) * len(out_names),
                  check_rep=False),
        donate_argnums=donate, keep_unused=True)
    sh = NamedSharding(mesh, PartitionSpec("core"))
    zeros_fn = jax.jit(lambda: jnp.zeros((FB * N, C + 4), jnp.int8),
                       out_shardings=sh)
    _CACHE["jitf"] = jitf
    _CACHE["sharding"] = sh
    _CACHE["zeros_fn"] = zeros_fn


class _Res:
    __slots__ = ("out", "results", "exec_time_ns", "mean_exec_time_ns",
                 "instructions_and_trace", "profile_json")

    def __init__(self, out):
        self.out = out                                   # [FB, N, C+4] int8
        self.results = [{"attnq": out[i]} for i in range(FB)]
        self.exec_time_ns = None
        self.mean_exec_time_ns = None
        self.instructions_and_trace = None
        self.profile_json = None


def _per_core_maps(in_):
    xq, dqk, wts = in_["xq"], in_["dqk"], in_["wts"]
    return [{"xq": xq[i * C:(i + 1) * C],
             "dqk": dqk[i * 2 * DQK:(i + 1) * 2 * DQK],
             "wts": wts[i * C:(i + 1) * C]} for i in range(FB)]


def _run(in_, trace=False):
    if trace or "spmd_done" not in _CACHE:
        # compile + run via bass_utils (first call / trace requests); warms
        # the persistent cache with the identical jit the fast path reuses.
        from concourse.bass_utils import run_bass_kernel_spmd
        res = run_bass_kernel_spmd(
            _get_nc(), _per_core_maps(in_), list(range(FB)),
            trace=trace, trace_cores=[0] if trace else None)
        _CACHE["spmd_done"] = True
        out = np.stack([res.results[i]["attnq"] for i in range(FB)], axis=0)
        r = _Res(out)
        r.exec_time_ns = res.exec_time_ns
        r.mean_exec_time_ns = getattr(res, "mean_exec_time_ns", None)
        r.instructions_and_trace = res.instructions_and_trace
        return r

    if "jitf" not in _CACHE:
        _build_fast()
    wts_dev = _CACHE.get("wts_dev")
    if wts_dev is None or not _CACHE.get("wts_dev_ok", False):
        wts_dev = jax.device_put(in_["wts"], _CACHE["sharding"])
        _CACHE["wts_dev"] = wts_dev
        _CACHE["wts_dev_ok"] = True
    donate_buf = _CACHE.pop("donate", None)
    if donate_buf is None:
        donate_buf = _CACHE["zeros_fn"]()
    outs = _CACHE["jitf"](in_["xq"], in_["dqk"], wts_dev, donate_buf)
    try:
        outs[0].copy_to_host_async()
    except Exception:
        pass
    flat = np.asarray(outs[0])                           # fetch
    _CACHE["donate"] = outs[0]
    return _Res(flat.reshape(FB, N, C + 4))


def _weights_np(Wq, bq, Wk, bk, Wv):
    """Build (and cache) the replicated fp16 weight tensor [FB*C, WCOLS]."""
    wc = _CACHE.get("wts_src")
    args = (Wq, bq, Wk, bk, Wv)
    if wc is not None and all(
            np.array_equal(a, b) for a, b in zip(wc, args)):
        return _CACHE["wts_np"]
    w = np.empty((C, WCOLS), np.float16)
    w[:, 0:DQK] = np.asarray(Wq, np.float32).T
    w[:, DQK:2 * DQK] = np.asarray(Wk, np.float32).T
    w[:, 2 * DQK:2 * DQK + C] = np.asarray(Wv, np.float32).T
    col = np.zeros((C,), np.float32)
    col[0:DQK] = np.asarray(bq, np.float32)
    col[DQK:2 * DQK] = np.asarray(bk, np.float32)
    w[:, WCOLS - 1] = col
    wts = np.tile(w, (FB, 1))
    _CACHE["wts_src"] = tuple(np.array(a, copy=True) for a in args)
    _CACHE["wts_np"] = wts
    _CACHE["wts_dev_ok"] = False                          # force re-upload
    # fp32 copies for the host-side q/k correction GEMM
    _CACHE["Wqk32"] = np.concatenate(
        [np.asarray(Wq, np.float32), np.asarray(Wk, np.float32)], axis=0)
    _CACHE["bqk32"] = np.concatenate(
        [np.asarray(bq, np.float32), np.asarray(bk, np.float32)])
    return wts


def _prep_inputs(features, Wq, bq, Wk, bk, Wv, bv, gamma):
    x_all = np.asarray(features, dtype=np.float32).reshape(FB * C, N)
    wts = _weights_np(Wq, bq, Wk, bk, Wv)
    buf = _CACHE.get("xq_buf")
    if buf is None:
        buf = _CACHE["xq_buf"] = np.empty((FB * C, PACKX), np.int8)
        _CACHE["tmp_f32"] = np.empty((FB * C, N), np.float32)
        _CACHE["x16_f16"] = np.empty((FB * C, N), np.float16)
        _CACHE["dq_f32"] = np.empty((FB, 2 * DQK, N), np.float32)
        _CACHE["dtmp_f32"] = np.empty((FB * 2 * DQK, N), np.float32)
        _CACHE["dq_buf"] = np.empty((FB * 2 * DQK, PACKX), np.int8)
    tmp, x16 = _CACHE["tmp_f32"], _CACHE["x16_f16"]
    dqs, dtmp, dbuf = _CACHE["dq_f32"], _CACHE["dtmp_f32"], _CACHE["dq_buf"]
    # per-(core,channel) symmetric int8 quantization of x
    np.abs(x_all, out=tmp)
    amax = tmp.max(axis=1)
    np.maximum(amax, 1e-20, out=amax)
    inv = np.float32(127.0) / amax
    np.multiply(x_all, inv[:, None], out=tmp)
    np.rint(tmp, out=tmp)
    np.copyto(buf[:, 0:N], tmp, casting="unsafe")
    scale = (amax * np.float32(1.0 / 127.0)).astype(np.float32)
    buf[:, N:N + 4] = scale.reshape(-1, 1).view(np.int8)
    # q/k correction: dqk = Wqk @ (x - fp16(dequant(x_int8))) + bqk, which
    # restores q/k to ~fp16 precision on top of the device's coarse path.
    np.multiply(tmp, scale[:, None], out=tmp)            # dequant, fp32
    np.copyto(x16, tmp)                                  # device-side fp16 rounding
    np.subtract(x_all, x16, out=tmp)                     # quantization error
    np.matmul(_CACHE["Wqk32"], tmp.reshape(FB, C, N), out=dqs)
    dqs += _CACHE["bqk32"][None, :, None]
    dflat = dqs.reshape(FB * 2 * DQK, N)
    np.abs(dflat, out=dtmp)
    damax = dtmp.max(axis=1)
    np.maximum(damax, 1e-20, out=damax)
    dinv = np.float32(127.0) / damax
    np.multiply(dflat, dinv[:, None], out=dtmp)
    np.rint(dtmp, out=dtmp)
    np.copyto(dbuf[:, 0:N], dtmp, casting="unsafe")
    dscale = (damax * np.float32(1.0 / 127.0)).astype(np.float32)
    dbuf[:, N:N + 4] = dscale.reshape(-1, 1).view(np.int8)
    return {"xq": buf, "dqk": dbuf, "wts": wts}


def kernel(features, Wq, bq, Wk, bk, Wv, bv, gamma):
    in_ = _prep_inputs(features, Wq, bq, Wk, bk, Wv, bv, gamma)
    res = _run(in_, trace=False)
    # device returns int8 attention with per-token fp32 scales bit-packed in
    # the last 4 columns; dequant + the epilogue gamma * (attn + bv) + x run
    # here in fp32.
    raw = res.out                                        # [FB, N, C+4] int8
    scales = np.ascontiguousarray(raw[:, :, C:C + 4]).view(np.float32)
    attn = raw[:, :, 0:C].astype(np.float32) * scales    # [FB, N, C]
    attn = attn.transpose(0, 2, 1)                       # [FB, C, N]
    x_all = np.asarray(features, dtype=np.float32).reshape(FB, C, N)
    g = np.float32(np.asarray(gamma, dtype=np.float32).reshape(-1)[0])
    bvv = np.asarray(bv, dtype=np.float32).reshape(1, C, 1)
    out = g * (attn + bvv) + x_all
    return out.reshape(F, B, C, HH, WW).astype(np.float32)


# revision 22
# speedup vs baseline: 1.6096x; 1.0091x over previous
"""CrossFrameAttention Trainium2 kernel.

Full (unsharded) inputs -> full output. Internally: data-parallel over the
fused frame*batch dim (F*B = 8 elements, one per NeuronCore), weights
replicated. Per core, a fused 1x1-conv QKV projection + softmax attention
written in Bass/Tile.

The warm wall-clock of a call is dominated by the axon tunnel (~70-90 MB/s
serial pipe with ~50 ms fixed upload cost and ~100 ms fixed fetch cost), so
the kernel is shaped around minimizing tunnel bytes and per-call overhead:
  - x ships as int8 with one fp32 scale per (core, channel) row, bit-packed
    into the same tensor (8.4 MB up instead of 16.8 MB fp16); the device
    upcasts to fp16 with the per-partition scale before the matmuls;
  - int8 noise on x would be amplified by the softmax (logits have std ~6,
    so 0.9% q/k noise moves dominant attention weights by ~7%), so the host
    also ships an int8 correction dq = Wqk @ (x - dequant(x_int8)) + b
    (+0.26 MB/core) that the device adds to its coarse q/k — q/k reach
    ~fp16 precision while x stays int8 for the V path, where the noise is
    benign;
  - projection weights ship in a separate small fp16 tensor that stays
    device-resident across calls (re-uploaded only if the weight bytes
    change);
  - the donated output buffer is the previous call's device-side output
    (the kernel writes every output byte), so no zero-buffer upload;
  - the jitted SPMD callable is built once and cached (no per-call
    re-trace), and the JAX persistent compilation cache makes the first
    call's XLA+NEFF compile a disk hit;
  - the attention output returns as int8 with per-token fp32 scales (the
    softmax denominator and the int8 dequant step are folded into one
    scale); the cheap epilogue gamma * (attn + bv) + x runs on the host
    in fp32.

Self-contained: hardcodes shapes from the problem spec.
"""

import numpy as np

import jax

# First-call XLA+NEFF compiles become disk hits across processes.
jax.config.update("jax_compilation_cache_dir", "/tmp/jax_bass_cc")
jax.config.update("jax_persistent_cache_min_compile_time_secs", 0.0)
jax.config.update("jax_persistent_cache_min_entry_size_bytes", -1)

F, B, C, HH, WW = 4, 2, 256, 64, 64
N = HH * WW            # 4096 tokens per (frame,batch) element
FB = F * B             # 8 == n_cores
DQK = 32               # q/k channel dim (C/8)
NBLK = N // 512        # 8 query blocks of 512
NJ = N // 128          # 32 key chunks of 128
PACKX = N + 4          # int8 x row + bit-packed fp32 scale
WCOLS = 2 * DQK + C + 1  # [WqT | WkT | WvT | bqk] columns, fp16

_CACHE = {}


def _build_nc():
    import concourse.mybir as mybir
    from concourse import bacc
    from concourse.tile import TileContext

    f32 = mybir.dt.float32
    f16 = mybir.dt.float16
    bf16 = mybir.dt.bfloat16
    i8 = mybir.dt.int8
    AF = mybir.ActivationFunctionType
    ALU = mybir.AluOpType

    nc = bacc.Bacc(None, target_bir_lowering=False, debug=False)

    # x int8 [C, N] with the per-channel fp32 dequant scale bit-packed into
    # the last 4 int8 columns (single per-call upload tensor).
    x_d = nc.dram_tensor("xq", [C, PACKX], i8, kind="ExternalInput")
    # q/k correction, int8 rows [q | k | -shift] with per-row fp32 scales
    # bit-packed into the last 4 columns; includes the bq/bk biases. The
    # last row is a per-query negative softmax shift: it lands in an
    # augmented q row that multiplies an all-ones k row, so the S matmul
    # emits pre-shifted logits and exp never overflows (softmax is
    # invariant to any per-query constant, so its quantization is harmless).
    dq_d = nc.dram_tensor("dqk", [2 * DQK + 1, PACKX], i8, kind="ExternalInput")
    # weights fp16, device-resident across calls: [WqT | WkT | WvT | bqk]
    w_d = nc.dram_tensor("wts", [C, WCOLS], f16, kind="ExternalInput")
    wqk_d = w_d[:, 0:2 * DQK]
    wv_d = w_d[:, 2 * DQK:2 * DQK + C]
    # attention output, [token, channel] layout, int8 with one fp32 scale
    # per token: attn = q * s. The softmax denominator is folded into s, so
    # the device never divides by it elementwise. The f32 scale is
    # bit-packed into the last 4 int8 columns.
    out_d = nc.dram_tensor("attnq", [N, C + 4], i8, kind="ExternalOutput")

    with TileContext(nc) as tc:
        with (
            tc.tile_pool(name="const", bufs=1) as cst,
            tc.tile_pool(name="x8p", bufs=1) as x8p,
            tc.tile_pool(name="xp", bufs=1) as xp,
            tc.tile_pool(name="qks", bufs=1) as qks,
            tc.tile_pool(name="vtp", bufs=1) as vtp,
            tc.tile_pool(name="ep", bufs=16) as ep,
            tc.tile_pool(name="aop", bufs=8) as aop,
            tc.tile_pool(name="rcp", bufs=8) as rcp,
            tc.tile_pool(name="ps_s", bufs=3, space="PSUM") as ps_s,
            tc.tile_pool(name="ps_av", bufs=4, space="PSUM") as ps_av,
        ):
            # ---- weights / biases ----
            wqk_t = [cst.tile([128, 2 * DQK], f16, tag=f"wqk{c}", name=f"wqk{c}")
                     for c in range(2)]
            wv_t = [cst.tile([128, C], f16, tag=f"wv{c}", name=f"wv{c}")
                    for c in range(2)]
            for c in range(2):
                nc.sync.dma_start(out=wqk_t[c], in_=wqk_d[c * 128:(c + 1) * 128, :])
                nc.sync.dma_start(out=wv_t[c], in_=wv_d[c * 128:(c + 1) * 128, :])
            ones_bt = cst.tile([128, 1], bf16, tag="ones", name="ones_bt")
            nc.gpsimd.memset(ones_bt, 1.0)

            # ---- q/k int8 correction rows -> fp16 [DQK, N] tiles ----
            dq8 = [cst.tile([DQK, N], i8, tag=f"dq8_{h}", name=f"dq8_{h}")
                   for h in range(2)]
            ds8 = [cst.tile([DQK, 4], i8, tag=f"ds8_{h}", name=f"ds8_{h}")
                   for h in range(2)]
            dqf = [cst.tile([DQK, N], f16, tag=f"dqf_{h}", name=f"dqf_{h}")
                   for h in range(2)]
            for h in range(2):
                nc.sync.dma_start(out=dq8[h],
                                  in_=dq_d[h * DQK:(h + 1) * DQK, 0:N])
                nc.sync.dma_start(out=ds8[h],
                                  in_=dq_d[h * DQK:(h + 1) * DQK, N:N + 4])
                nc.scalar.activation(dqf[h], dq8[h], AF.Identity,
                                     scale=ds8[h].bitcast(f32))
            sh8 = cst.tile([1, N], i8, tag="sh8", name="sh8")
            shs8 = cst.tile([1, 4], i8, tag="shs8", name="shs8")
            nc.sync.dma_start(out=sh8, in_=dq_d[2 * DQK:2 * DQK + 1, 0:N])
            nc.sync.dma_start(out=shs8, in_=dq_d[2 * DQK:2 * DQK + 1, N:N + 4])

            # ---- x: int8 tiles + per-channel scale -> fp16 tiles ----
            s8 = [cst.tile([128, 4], i8, tag=f"s8_{c}", name=f"s8_{c}")
                  for c in range(2)]
            for c in range(2):
                nc.sync.dma_start(out=s8[c],
                                  in_=x_d[c * 128:(c + 1) * 128, N:N + 4])
            sf = [s8[c].bitcast(f32) for c in range(2)]

            x8_t = [[x8p.tile([128, 512], i8, tag=f"x8_{c}_{nb}",
                              name=f"x8_{c}_{nb}") for nb in range(NBLK)]
                    for c in range(2)]
            x_t = [[xp.tile([128, 512], f16, tag=f"x{c}_{nb}", name=f"x{c}_{nb}")
                    for nb in range(NBLK)] for c in range(2)]
            for nb in range(NBLK):
                for c in range(2):
                    nc.sync.dma_start(
                        out=x8_t[c][nb],
                        in_=x_d[c * 128:(c + 1) * 128, nb * 512:(nb + 1) * 512])
                    nc.scalar.activation(x_t[c][nb], x8_t[c][nb], AF.Identity,
                                         scale=sf[c])

            # q/k with one augmented contraction row: q row DQK = -shift,
            # k row DQK = 1.0, so S = k^T q comes out pre-shifted per query.
            q_sb = qks.tile([DQK + 1, N], f16, tag="q", name="q_sb")
            k_sb = qks.tile([DQK + 1, N], f16, tag="k", name="k_sb")
            nc.scalar.activation(q_sb[DQK:DQK + 1, :], sh8, AF.Identity,
                                 scale=shs8.bitcast(f32))
            nc.gpsimd.memset(k_sb[DQK:DQK + 1, :], 1.0)

            # ---- QK projection: q = Wq @ x + dq, k = Wk @ x + dk ----
            # (bias is folded into the host-computed dq/dk correction)
            for nb in range(NBLK):
                blk = slice(nb * 512, (nb + 1) * 512)
                q_ps = ps_s.tile([DQK, 512], f32, tag="s", name=f"qps{nb}")
                nc.tensor.matmul(q_ps, lhsT=wqk_t[0][:, 0:DQK],
                                 rhs=x_t[0][nb], start=True, stop=False)
                nc.tensor.matmul(q_ps, lhsT=wqk_t[1][:, 0:DQK],
                                 rhs=x_t[1][nb], start=False, stop=True)
                nc.vector.scalar_tensor_tensor(
                    q_sb[0:DQK, blk], q_ps, 1.0, dqf[0][:, blk],
                    ALU.mult, ALU.add)
                k_ps = ps_s.tile([DQK, 512], f32, tag="s", name=f"kps{nb}")
                nc.tensor.matmul(k_ps, lhsT=wqk_t[0][:, DQK:2 * DQK],
                                 rhs=x_t[0][nb], start=True, stop=False)
                nc.tensor.matmul(k_ps, lhsT=wqk_t[1][:, DQK:2 * DQK],
                                 rhs=x_t[1][nb], start=False, stop=True)
                nc.vector.scalar_tensor_tensor(
                    k_sb[0:DQK, blk], k_ps, 1.0, dqf[1][:, blk],
                    ALU.mult, ALU.add)

            # ---- V projection, directly transposed: vT[j, c] = x[:, j].T @ WvT
            # vT tiles [128 (j), 257]; col 256 = 1.0 so the AV matmul also
            # produces sum_j(E) ("ones trick") for the softmax denominator.
            vt_t = []
            for j in range(NJ):
                nb, off = divmod(j * 128, 512)
                pv = ps_av.tile([128, C], f32, tag="av", name=f"vps{j}")
                nc.tensor.matmul(pv, lhsT=x_t[0][nb][:, off:off + 128],
                                 rhs=wv_t[0], start=True, stop=False)
                nc.tensor.matmul(pv, lhsT=x_t[1][nb][:, off:off + 128],
                                 rhs=wv_t[1], start=False, stop=True)
                vt = vtp.tile([128, C + 1], bf16, tag=f"vt{j}", name=f"vt{j}")
                nc.scalar.activation(vt[:, 0:C], pv, AF.Copy)
                nc.scalar.activation(vt[:, C:C + 1], ones_bt, AF.Copy)
                vt_t.append(vt)

            # ---- attention over 8 query blocks of 512 ----
            # logits arrive pre-shifted by the host's per-query shift (the
            # augmented q/k row), so exp never overflows; softmax is
            # invariant to the per-query constant.
            for ib in range(NBLK):
                av_ps = [ps_av.tile([128, C + 1], f32, tag="av", name=f"av{ib}_{q}")
                         for q in range(4)]
                e_t = {}
                for j in range(NJ):
                    s_ps = ps_s.tile([128, 512], f32, tag="s", name=f"sps{ib}_{j}")
                    nc.tensor.matmul(
                        s_ps, lhsT=k_sb[:, j * 128:(j + 1) * 128],
                        rhs=q_sb[:, ib * 512:(ib + 1) * 512],
                        start=True, stop=True)
                    et = ep.tile([128, 512], bf16, tag="e", name=f"e{ib}_{j}")
                    nc.scalar.activation(et, s_ps, AF.Exp)
                    e_t[j] = et
                    if j >= 2:
                        jj = j - 2
                        for q in range(4):
                            nc.tensor.matmul(
                                av_ps[q], lhsT=e_t[jj][:, q * 128:(q + 1) * 128],
                                rhs=vt_t[jj], start=(jj == 0), stop=False)
                for jj in (NJ - 2, NJ - 1):
                    for q in range(4):
                        nc.tensor.matmul(
                            av_ps[q], lhsT=e_t[jj][:, q * 128:(q + 1) * 128],
                            rhs=vt_t[jj], start=False, stop=(jj == NJ - 1))

                # int8 quantization: q = av * (127/maxabs(av)); the softmax
                # 1/sumexp and the maxabs/127 dequant step both land in the
                # per-token scale s = maxabs * (1/sumexp) / 127.
                for q in range(4):
                    mx = rcp.tile([128, 1], f32, tag="mx", name=f"mx{ib}_{q}")
                    nc.vector.tensor_reduce(
                        mx, av_ps[q][:, 0:C], axis=mybir.AxisListType.X,
                        op=ALU.max, apply_absolute_value=True)
                    mxc = rcp.tile([128, 1], f32, tag="mxc", name=f"mxc{ib}_{q}")
                    nc.vector.tensor_scalar(mxc, mx, 1e-30, None, ALU.max)
                    rq = rcp.tile([128, 1], f32, tag="rq", name=f"rq{ib}_{q}")
                    nc.vector.reciprocal(rq, mxc)
                    ao = aop.tile([128, C], i8, tag="ao", name=f"ao{ib}_{q}")
                    nc.vector.tensor_scalar(ao, av_ps[q][:, 0:C], rq, 127.0,
                                            ALU.mult, ALU.mult)
                    r0 = ib * 512 + q * 128
                    nc.sync.dma_start(out=out_d[r0:r0 + 128, 0:C], in_=ao)
                    rs = rcp.tile([128, 1], f32, tag="rs", name=f"rs{ib}_{q}")
                    nc.vector.reciprocal(rs, av_ps[q][:, C:C + 1])
                    st = rcp.tile([128, 1], f32, tag="st", name=f"st{ib}_{q}")
                    nc.vector.tensor_scalar(st, mxc, rs, 1.0 / 127.0,
                                            ALU.mult, ALU.mult)
                    nc.sync.dma_start(out=out_d[r0:r0 + 128, C:C + 4],
                                      in_=st.bitcast(i8))

    nc.finalize()
    return nc


def _get_nc():
    if "nc" not in _CACHE:
        _CACHE["nc"] = _build_nc()
    return _CACHE["nc"]


def _build_fast():
    """One-time setup of the cached jitted SPMD callable (replicates
    bass_utils.run_bass_kernel_spmd's axon path, hoisted out of the
    per-call loop so warm calls skip the re-trace)."""
    import jax.numpy as jnp
    from jax.sharding import Mesh, PartitionSpec, NamedSharding
    from jax.experimental.shard_map import shard_map
    from concourse import bass2jax
    from concourse.bass2jax import _bass_exec_p, install_neuronx_cc_hook
    import concourse.mybir as mybir

    install_neuronx_cc_hook()
    nc = _get_nc()

    partition_name = (nc.partition_id_tensor.name
                      if nc.partition_id_tensor else None)
    in_names, out_names, out_avals = [], [], []
    for alloc in nc.m.functions[0].allocations:
        if not isinstance(alloc, mybir.MemoryLocationSet):
            continue
        name = alloc.memorylocations[0].name
        if alloc.kind == "ExternalInput":
            if name != partition_name:
                in_names.append(name)
        elif alloc.kind == "ExternalOutput":
            out_names.append(name)
            out_avals.append(jax.core.ShapedArray(
                tuple(alloc.tensor_shape), mybir.dt.np(alloc.dtype)))
    assert in_names == ["xq", "dqk", "wts"] and out_names == ["attnq"], (
        in_names, out_names)
    n_params = len(in_names)
    all_in = in_names + out_names
    if partition_name is not None:
        all_in.append(partition_name)
    donate = tuple(range(n_params, n_params + len(out_names)))

    def _body(*args):
        operands = list(args)
        if partition_name is not None:
            operands.append(bass2jax.partition_id_tensor())
        outs = _bass_exec_p.bind(
            *operands, out_avals=tuple(out_avals), in_names=tuple(all_in),
            out_names=tuple(out_names), lowering_input_output_aliases=(),
            sim_require_finite=True, sim_require_nnan=True, nc=nc)
        return tuple(outs)

    devices = jax.devices()[:FB]
    mesh = Mesh(np.asarray(devices), ("core",))
    n_args = n_params + len(out_names)
    jitf = jax.jit(
        shard_map(_body, mesh=mesh,
                  in_specs=(PartitionSpec("core"),) * n_args,
                  out_specs=(PartitionSpec("core"),) * len(out_names),
                  check_rep=False),
        donate_argnums=donate, keep_unused=True)
    sh = NamedSharding(mesh, PartitionSpec("core"))
    zeros_fn = jax.jit(lambda: jnp.zeros((FB * N, C + 4), jnp.int8),
                       out_shardings=sh)
    _CACHE["jitf"] = jitf
    _CACHE["sharding"] = sh
    _CACHE["zeros_fn"] = zeros_fn


class _Res:
    __slots__ = ("out", "results", "exec_time_ns", "mean_exec_time_ns",
                 "instructions_and_trace", "profile_json")

    def __init__(self, out):
        self.out = out                                   # [FB, N, C+4] int8
        self.results = [{"attnq": out[i]} for i in range(FB)]
        self.exec_time_ns = None
        self.mean_exec_time_ns = None
        self.instructions_and_trace = None
        self.profile_json = None


DROW = 2 * DQK + 1     # dqk tensor rows per core: [dq | dk | -shift]


def _per_core_maps(in_):
    xq, dqk, wts = in_["xq"], in_["dqk"], in_["wts"]
    return [{"xq": xq[i * C:(i + 1) * C],
             "dqk": dqk[i * DROW:(i + 1) * DROW],
             "wts": wts[i * C:(i + 1) * C]} for i in range(FB)]


def _run(in_, trace=False):
    if trace or "spmd_done" not in _CACHE:
        # compile + run via bass_utils (first call / trace requests); warms
        # the persistent cache with the identical jit the fast path reuses.
        from concourse.bass_utils import run_bass_kernel_spmd
        res = run_bass_kernel_spmd(
            _get_nc(), _per_core_maps(in_), list(range(FB)),
            trace=trace, trace_cores=[0] if trace else None)
        _CACHE["spmd_done"] = True
        out = np.stack([res.results[i]["attnq"] for i in range(FB)], axis=0)
        r = _Res(out)
        r.exec_time_ns = res.exec_time_ns
        r.mean_exec_time_ns = getattr(res, "mean_exec_time_ns", None)
        r.instructions_and_trace = res.instructions_and_trace
        return r

    if "jitf" not in _CACHE:
        _build_fast()
    wts_dev = _CACHE.get("wts_dev")
    if wts_dev is None or not _CACHE.get("wts_dev_ok", False):
        wts_dev = jax.device_put(in_["wts"], _CACHE["sharding"])
        _CACHE["wts_dev"] = wts_dev
        _CACHE["wts_dev_ok"] = True
    donate_buf = _CACHE.pop("donate", None)
    if donate_buf is None:
        donate_buf = _CACHE["zeros_fn"]()
    outs = _CACHE["jitf"](in_["xq"], in_["dqk"], wts_dev, donate_buf)
    try:
        outs[0].copy_to_host_async()
    except Exception:
        pass
    flat = np.asarray(outs[0])                           # fetch
    _CACHE["donate"] = outs[0]
    return _Res(flat.reshape(FB, N, C + 4))


def _weights_np(Wq, bq, Wk, bk, Wv):
    """Build (and cache) the replicated fp16 weight tensor [FB*C, WCOLS]."""
    wc = _CACHE.get("wts_src")
    args = (Wq, bq, Wk, bk, Wv)
    if wc is not None and all(
            np.array_equal(a, b) for a, b in zip(wc, args)):
        return _CACHE["wts_np"]
    w = np.empty((C, WCOLS), np.float16)
    w[:, 0:DQK] = np.asarray(Wq, np.float32).T
    w[:, DQK:2 * DQK] = np.asarray(Wk, np.float32).T
    w[:, 2 * DQK:2 * DQK + C] = np.asarray(Wv, np.float32).T
    col = np.zeros((C,), np.float32)
    col[0:DQK] = np.asarray(bq, np.float32)
    col[DQK:2 * DQK] = np.asarray(bk, np.float32)
    w[:, WCOLS - 1] = col
    wts = np.tile(w, (FB, 1))
    _CACHE["wts_src"] = tuple(np.array(a, copy=True) for a in args)
    _CACHE["wts_np"] = wts
    _CACHE["wts_dev_ok"] = False                          # force re-upload
    # fp32 copies for the host-side q/k correction GEMM
    _CACHE["Wqk32"] = np.concatenate(
        [np.asarray(Wq, np.float32), np.asarray(Wk, np.float32)], axis=0)
    _CACHE["bqk32"] = np.concatenate(
        [np.asarray(bq, np.float32), np.asarray(bk, np.float32)])
    return wts


def _prep_inputs(features, Wq, bq, Wk, bk, Wv, bv, gamma):
    x_all = np.asarray(features, dtype=np.float32).reshape(FB * C, N)
    wts = _weights_np(Wq, bq, Wk, bk, Wv)
    buf = _CACHE.get("xq_buf")
    if buf is None:
        buf = _CACHE["xq_buf"] = np.empty((FB * C, PACKX), np.int8)
        _CACHE["tmp_f32"] = np.empty((FB * C, N), np.float32)
        _CACHE["x16_f16"] = np.empty((FB * C, N), np.float16)
        _CACHE["dq_f32"] = np.empty((FB, 2 * DQK, N), np.float32)
        _CACHE["qkf_f32"] = np.empty((FB, 2 * DQK, N), np.float32)
        _CACHE["dtmp_f32"] = np.empty((FB * 2 * DQK, N), np.float32)
        _CACHE["dq_buf"] = np.empty((FB * DROW, PACKX), np.int8)
    tmp, x16 = _CACHE["tmp_f32"], _CACHE["x16_f16"]
    dqs, dtmp, dbuf = _CACHE["dq_f32"], _CACHE["dtmp_f32"], _CACHE["dq_buf"]
    qkf = _CACHE["qkf_f32"]
    # per-(core,channel) symmetric int8 quantization of x
    np.abs(x_all, out=tmp)
    amax = tmp.max(axis=1)
    np.maximum(amax, 1e-20, out=amax)
    inv = np.float32(127.0) / amax
    np.multiply(x_all, inv[:, None], out=tmp)
    np.rint(tmp, out=tmp)
    np.copyto(buf[:, 0:N], tmp, casting="unsafe")
    scale = (amax * np.float32(1.0 / 127.0)).astype(np.float32)
    buf[:, N:N + 4] = scale.reshape(-1, 1).view(np.int8)
    # q/k correction: dqk = Wqk @ (x - fp16(dequant(x_int8))) + bqk, which
    # restores q/k to ~fp16 precision on top of the device's coarse path.
    np.multiply(tmp, scale[:, None], out=tmp)            # dequant, fp32
    np.copyto(x16, tmp)                                  # device-side fp16 rounding
    np.subtract(x_all, x16, out=tmp)                     # quantization error
    np.matmul(_CACHE["Wqk32"], tmp.reshape(FB, C, N), out=dqs)
    dqs += _CACHE["bqk32"][None, :, None]
    dflat = dqs.reshape(FB * 2 * DQK, N)
    dbuf3 = dbuf.reshape(FB, DROW, PACKX)
    np.abs(dflat, out=dtmp)
    damax = dtmp.max(axis=1)
    np.maximum(damax, 1e-20, out=damax)
    dinv = np.float32(127.0) / damax
    np.multiply(dflat, dinv[:, None], out=dtmp)
    np.rint(dtmp, out=dtmp)
    np.copyto(dbuf3[:, 0:2 * DQK, 0:N],
              dtmp.reshape(FB, 2 * DQK, N), casting="unsafe")
    dscale = (damax * np.float32(1.0 / 127.0)).astype(np.float32)
    dbuf3[:, 0:2 * DQK, N:N + 4] = (
        dscale.reshape(FB, 2 * DQK, 1).view(np.int8))
    # per-query softmax shift: at least the max over a key sample, and at
    # least (||q||*max||k|| - 60) so exp's argument stays under ~61 for any
    # input magnitude. Softmax is invariant to the shift's exact value.
    np.matmul(_CACHE["Wqk32"], x_all.reshape(FB, C, N), out=qkf)
    qkf += _CACHE["bqk32"][None, :, None]
    qf, kf = qkf[:, 0:DQK], qkf[:, DQK:2 * DQK]
    smax = np.matmul(qf.transpose(0, 2, 1), kf[:, :, ::32]).max(axis=2)
    qn = np.linalg.norm(qf, axis=1)                      # [FB, N]
    kmax = np.linalg.norm(kf, axis=1).max(axis=1)        # [FB]
    shift = np.maximum(smax, qn * kmax[:, None] - np.float32(60.0))
    samax = np.maximum(np.abs(shift).max(axis=1), 1e-20)
    sinv = (np.float32(127.0) / samax)[:, None]
    dbuf3[:, 2 * DQK, 0:N] = np.rint(-shift * sinv).astype(np.int8)
    sscale = (samax * np.float32(1.0 / 127.0)).astype(np.float32)
    dbuf3[:, 2 * DQK, N:N + 4] = sscale.reshape(FB, 1).view(np.int8)
    return {"xq": buf, "dqk": dbuf, "wts": wts}


def kernel(features, Wq, bq, Wk, bk, Wv, bv, gamma):
    in_ = _prep_inputs(features, Wq, bq, Wk, bk, Wv, bv, gamma)
    res = _run(in_, trace=False)
    # device returns int8 attention with per-token fp32 scales bit-packed in
    # the last 4 columns; dequant + the epilogue gamma * (attn + bv) + x run
    # here in fp32.
    raw = res.out                                        # [FB, N, C+4] int8
    scales = np.ascontiguousarray(raw[:, :, C:C + 4]).view(np.float32)
    attn = raw[:, :, 0:C].astype(np.float32) * scales    # [FB, N, C]
    attn = attn.transpose(0, 2, 1)                       # [FB, C, N]
    x_all = np.asarray(features, dtype=np.float32).reshape(FB, C, N)
    g = np.float32(np.asarray(gamma, dtype=np.float32).reshape(-1)[0])
    bvv = np.asarray(bv, dtype=np.float32).reshape(1, C, 1)
    out = g * (attn + bvv) + x_all
    return out.reshape(F, B, C, HH, WW).astype(np.float32)
